# revision 1
# baseline (speedup 1.0000x reference)
"""Trainium2 Bass kernel for nn_LSH: ret[o] = sum_{s,a} x[s] * w[o,s,a].

x: [1, 4096] f32, weights: [512, 4096, 128] f32 -> ret: [512] f32.

Sharding: out_dim 512 is split 64-per-core across 8 cores; x is replicated.
Per core the 64x4096x128 f32 slice (128 MiB) is streamed from HBM as a flat
[128, 262144] layout (partition p = o=p//2, s in [(p%2)*2048, ...+2048)).
Compute per chunk: DVE segmented reduce over the innermost a=128 giving
T[p, s_local]; partial x-multiply+reduce stages overlap the stream; a tiny
pairing matmul folds partition pairs (2o, 2o+1) into ret[o].
The tail chunks taper down so the last DVE reduce is short.
"""

import sys

sys.path.insert(0, "/opt/trn_rl_repo")

import numpy as np

import concourse.bass as bass
import concourse.mybir as mybir
import concourse.tile as tile
from concourse import bacc
from concourse.bass_utils import run_bass_kernel_spmd

P = 128
O_PER_CORE = 64
N_CORES = 8
S = 4096
A = 128
COLS = O_PER_CORE * S * A // P  # 262144 per-partition row length
SLOC = 2048  # s-values covered by each partition

# Chunk schedule: full 4 MiB DMAs for max bandwidth; the final chunk is
# split into 4 sub-DMAs (1 MiB each) so its DVE reduces overlap the tail.
CHUNKS = [8192] * 31 + [4096, 2048, 2048]
assert sum(CHUNKS) == COLS
# After these chunk indices, run a partial x-multiply+reduce stage.
PARTIAL_AFTER = [7, 15, 23, 29, 32, 33]
NPART = len(PARTIAL_AFTER)

_CACHED_NC = None


def _build_nc():
    nc = bacc.Bacc(
        "TRN2",
        target_bir_lowering=False,
        debug=False,
        num_devices=N_CORES,
    )
    w = nc.dram_tensor("w", [P, COLS], mybir.dt.float32, kind="ExternalInput").ap()
    xt = nc.dram_tensor("xt", [P, SLOC], mybir.dt.float32, kind="ExternalInput").ap()
    pmat = nc.dram_tensor(
        "pmat", [P, O_PER_CORE], mybir.dt.float32, kind="ExternalInput"
    ).ap()
    out = nc.dram_tensor(
        "out", [O_PER_CORE, 1], mybir.dt.float32, kind="ExternalOutput"
    ).ap()

    with tile.TileContext(nc) as tc:
        with (
            tc.tile_pool(name="wp", bufs=3) as wp,
            tc.tile_pool(name="const", bufs=1) as constp,
            tc.tile_pool(name="accp", bufs=1) as accp,
            tc.tile_pool(name="psum", bufs=1, space="PSUM") as psp,
        ):
            acc = accp.tile([P, SLOC], mybir.dt.float32)
            accx = accp.tile([P, SLOC], mybir.dt.float32)
            vparts = accp.tile([P, NPART], mybir.dt.float32)
            xt_t = constp.tile([P, SLOC], mybir.dt.float32)
            pm_t = constp.tile([P, O_PER_CORE], mybir.dt.float32)

            coff = 0  # acc column offset (completed s-values)
            boundaries = []  # acc col ranges per partial stage
            pstart = 0
            pi = 0
            for k, cols in enumerate(CHUNKS):
                wt = wp.tile([P, max(CHUNKS)], mybir.dt.float32, tag="wt")
                nseg = cols // A
                nc.sync.dma_start(
                    wt[:, :cols], w[:, coff * A : coff * A + cols]
                )
                if k == 1:
                    # Constants go via SWDGE so the HWDGE queue carries
                    # only the weight stream.
                    nc.gpsimd.dma_start(xt_t[:], xt[:])
                    nc.gpsimd.dma_start(pm_t[:], pmat[:])
                seg = wt[:, :cols].rearrange("p (n a) -> p n a", a=A)
                nc.vector.tensor_reduce(
                    acc[:, coff : coff + nseg],
                    seg,
                    axis=mybir.AxisListType.X,
                    op=mybir.AluOpType.add,
                )
                coff += nseg
                if k == PARTIAL_AFTER[pi]:
                    nc.vector.tensor_mul(
                        accx[:, pstart:coff], acc[:, pstart:coff], xt_t[:, pstart:coff]
                    )
                    nc.vector.tensor_reduce(
                        vparts[:, pi : pi + 1],
                        accx[:, pstart:coff],
                        axis=mybir.AxisListType.X,
                        op=mybir.AluOpType.add,
                    )
                    boundaries.append((pstart, coff))
                    pstart = coff
                    pi += 1
            assert coff == SLOC and pi == NPART

            v = accp.tile([P, 1], mybir.dt.float32)
            nc.vector.tensor_reduce(
                v[:], vparts[:], axis=mybir.AxisListType.X, op=mybir.AluOpType.add
            )
            ps = psp.tile([O_PER_CORE, 1], mybir.dt.float32)
            nc.tensor.matmul(ps[:], pm_t[:], v[:], start=True, stop=True)
            res = accp.tile([O_PER_CORE, 1], mybir.dt.float32)
            nc.scalar.copy(res[:], ps[:])
            nc.sync.dma_start(out[:], res[:])

    nc.compile()
    return nc


def _get_nc():
    global _CACHED_NC
    if _CACHED_NC is None:
        _CACHED_NC = _build_nc()
    return _CACHED_NC


def _in_maps(x, weights):
    x = np.ascontiguousarray(np.asarray(x, dtype=np.float32))
    weights = np.asarray(weights, dtype=np.float32)
    xt = np.tile(x.reshape(2, SLOC), (P // 2, 1))
    pmat = np.zeros((P, O_PER_CORE), dtype=np.float32)
    pmat[np.arange(P), np.arange(P) // 2] = 1.0
    maps = []
    for c in range(N_CORES):
        wc = np.ascontiguousarray(
            weights[c * O_PER_CORE : (c + 1) * O_PER_CORE]
        ).reshape(P, COLS)
        maps.append({"w": wc, "xt": xt, "pmat": pmat})
    return maps


def run(x, weights, trace=False):
    """Run on hardware; returns (ret[512], BassKernelResults)."""
    nc = _get_nc()
    res = run_bass_kernel_spmd(
        nc, _in_maps(x, weights), list(range(N_CORES)), trace=trace
    )
    ret = np.concatenate(
        [res.results[c]["out"].reshape(O_PER_CORE) for c in range(N_CORES)]
    ).astype(np.float32)
    return ret, res


def kernel(x, weights):
    ret, _ = run(x, weights)
    return ret



# revision 2
# speedup vs baseline: 1.4269x; 1.4269x over previous
"""Trainium2 Bass kernel for nn_LSH: ret[o] = sum_{s,a} x[s] * w[o,s,a].

x: [1, 4096] f32, weights: [512, 4096, 128] f32 -> ret: [512] f32.

Sharding: out_dim 512 is split 64-per-core across 8 cores; x is replicated.
Per core the 64x4096x128 f32 slice (128 MiB) is streamed from HBM.

Layout: the logical per-core view is [128 partitions, 262144 cols] with
partition p = o*2 + s//2048, col = (s%2048)*128 + a.  In HBM the stream is
re-packed chunk-major so every chunk DMA reads one fully CONTIGUOUS block
(descriptors walk sequential DRAM addresses; the partition-major layout's
1 MiB-strided descriptors only reached ~52% of per-engine DMA bandwidth).

Chunks: 2 head-taper DMAs, 19 x 6 MiB body DMAs (48 KiB/partition
descriptors), 4 tail-taper DMAs, alternating between the two HWDGE queues
(sync + scalar) so both rings keep the 16 SDMA engines fed.

Compute per chunk: DVE segmented reduce over the innermost a=128 giving
acc[p, s_local]; periodic x-multiply+reduce partial stages overlap the
stream; a tiny pairing matmul folds partition pairs (2o, 2o+1) into ret[o].
"""

import sys

sys.path.insert(0, "/opt/trn_rl_repo")

import numpy as np

import concourse.bass as bass
import concourse.mybir as mybir
import concourse.tile as tile
from concourse import bacc
from concourse.bass_utils import run_bass_kernel_spmd

P = 128
O_PER_CORE = 64
N_CORES = 8
S = 4096
A = 128
COLS = O_PER_CORE * S * A // P  # 262144 per-partition row length
SLOC = 2048  # s-values covered by each partition

# Chunk schedule (cols each; 2048 cols = 1 MiB). Head/tail taper so the
# first reduce starts early and the last reduce is short.
CHUNKS = [4096, 8192] + [12288] * 19 + [8192, 4096, 2048, 2048]
assert sum(CHUNKS) == COLS
# After these chunk indices, run a partial x-multiply+reduce stage.
PARTIAL_AFTER = [5, 10, 15, 20, 22, 23, 24]
NPART = len(PARTIAL_AFTER)

# DRAM tensor per chunk-size class: name -> (n_chunks, cols)
SIZE_CLASSES = {
    "w2k": (2, 2048),
    "w4k": (2, 4096),
    "w8k": (2, 8192),
    "w12k": (19, 12288),
}
# chunk index -> (tensor name, slot within tensor)
CHUNK_SRC = (
    [("w4k", 0), ("w8k", 0)]
    + [("w12k", k) for k in range(19)]
    + [("w8k", 1), ("w4k", 1), ("w2k", 0), ("w2k", 1)]
)

_CACHED_NC = None


def _build_nc():
    nc = bacc.Bacc(
        "TRN2",
        target_bir_lowering=False,
        debug=False,
        num_devices=N_CORES,
    )
    wts = {
        name: nc.dram_tensor(
            name, [n * P, cols], mybir.dt.float32, kind="ExternalInput"
        ).ap()
        for name, (n, cols) in SIZE_CLASSES.items()
    }
    xt = nc.dram_tensor("xt", [P, SLOC], mybir.dt.float32, kind="ExternalInput").ap()
    pmat = nc.dram_tensor(
        "pmat", [P, O_PER_CORE], mybir.dt.float32, kind="ExternalInput"
    ).ap()
    out = nc.dram_tensor(
        "out", [O_PER_CORE, 1], mybir.dt.float32, kind="ExternalOutput"
    ).ap()

    with tile.TileContext(nc) as tc:
        with (
            tc.tile_pool(name="wp", bufs=3) as wp,
            tc.tile_pool(name="const", bufs=1) as constp,
            tc.tile_pool(name="accp", bufs=1) as accp,
            tc.tile_pool(name="psum", bufs=1, space="PSUM") as psp,
        ):
            acc = accp.tile([P, SLOC], mybir.dt.float32)
            accx = accp.tile([P, SLOC], mybir.dt.float32)
            vparts = accp.tile([P, NPART], mybir.dt.float32)
            xt_t = constp.tile([P, SLOC], mybir.dt.float32)
            pm_t = constp.tile([P, O_PER_CORE], mybir.dt.float32)

            # Constants go via SWDGE so the HWDGE queues carry only the
            # weight stream.
            nc.gpsimd.dma_start(xt_t[:], xt[:])
            nc.gpsimd.dma_start(pm_t[:], pmat[:])

            coff = 0  # acc column offset (completed s-values)
            pstart = 0
            pi = 0
            for k, cols in enumerate(CHUNKS):
                wt = wp.tile([P, max(CHUNKS)], mybir.dt.float32, tag="wt")
                nseg = cols // A
                name, slot = CHUNK_SRC[k]
                src = wts[name][slot * P : (slot + 1) * P, :]
                eng = nc.sync if k % 2 == 0 else nc.scalar
                eng.dma_start(wt[:, :cols], src)
                seg = wt[:, :cols].rearrange("p (n a) -> p n a", a=A)
                nc.vector.tensor_reduce(
                    acc[:, coff : coff + nseg],
                    seg,
                    axis=mybir.AxisListType.X,
                    op=mybir.AluOpType.add,
                )
                coff += nseg
                if k == PARTIAL_AFTER[pi]:
                    nc.vector.tensor_mul(
                        accx[:, pstart:coff], acc[:, pstart:coff], xt_t[:, pstart:coff]
                    )
                    nc.vector.tensor_reduce(
                        vparts[:, pi : pi + 1],
                        accx[:, pstart:coff],
                        axis=mybir.AxisListType.X,
                        op=mybir.AluOpType.add,
                    )
                    pstart = coff
                    pi += 1
            assert coff == SLOC and pi == NPART

            v = accp.tile([P, 1], mybir.dt.float32)
            nc.vector.tensor_reduce(
                v[:], vparts[:], axis=mybir.AxisListType.X, op=mybir.AluOpType.add
            )
            ps = psp.tile([O_PER_CORE, 1], mybir.dt.float32)
            nc.tensor.matmul(ps[:], pm_t[:], v[:], start=True, stop=True)
            res = accp.tile([O_PER_CORE, 1], mybir.dt.float32)
            nc.scalar.copy(res[:], ps[:])
            nc.sync.dma_start(out[:], res[:])

    nc.compile()
    return nc


def _get_nc():
    global _CACHED_NC
    if _CACHED_NC is None:
        _CACHED_NC = _build_nc()
    return _CACHED_NC


def _in_maps(x, weights):
    x = np.ascontiguousarray(np.asarray(x, dtype=np.float32))
    weights = np.asarray(weights, dtype=np.float32)
    xt = np.tile(x.reshape(2, SLOC), (P // 2, 1))
    pmat = np.zeros((P, O_PER_CORE), dtype=np.float32)
    pmat[np.arange(P), np.arange(P) // 2] = 1.0

    # Column ranges per chunk in the logical [P, COLS] view.
    offs = np.cumsum([0] + CHUNKS)
    maps = []
    for c in range(N_CORES):
        wc = np.ascontiguousarray(
            weights[c * O_PER_CORE : (c + 1) * O_PER_CORE]
        ).reshape(P, COLS)
        m = {"xt": xt, "pmat": pmat}
        arrs = {
            name: np.empty((n * P, cols), dtype=np.float32)
            for name, (n, cols) in SIZE_CLASSES.items()
        }
        for k, cols in enumerate(CHUNKS):
            name, slot = CHUNK_SRC[k]
            arrs[name][slot * P : (slot + 1) * P, :] = wc[:, offs[k] : offs[k + 1]]
        m.update(arrs)
        maps.append(m)
    return maps


def run(x, weights, trace=False):
    """Run on hardware; returns (ret[512], BassKernelResults)."""
    nc = _get_nc()
    res = run_bass_kernel_spmd(
        nc, _in_maps(x, weights), list(range(N_CORES)), trace=trace
    )
    ret = np.concatenate(
        [res.results[c]["out"].reshape(O_PER_CORE) for c in range(N_CORES)]
    ).astype(np.float32)
    return ret, res


def kernel(x, weights):
    ret, _ = run(x, weights)
    return ret


# revision 5
# speedup vs baseline: 1.9499x; 1.3665x over previous
"""Trainium2 Bass kernel for nn_LSH: ret[o] = sum_{s,a} x[s] * w[o,s,a].

x: [1, 4096] f32, weights: [512, 4096, 128] f32 -> ret: [512] f32.

Sharding: out_dim 512 is split 64-per-core across 8 cores; x is replicated.
The 2e-2 tolerance admits bf16 weights, so the host casts w (and x) to
bf16 and each core streams a 64 MiB slice instead of 128 MiB -- the kernel
is HBM-bandwidth-bound, so this halves the roofline.

Layout: the logical per-core view is [128 partitions, 262144 cols] with
partition p = o*2 + s//2048, col = (s%2048)*128 + a.  In HBM the stream is
re-packed chunk-major so every chunk DMA reads one fully CONTIGUOUS block
(descriptors walk sequential DRAM addresses; a partition-major layout's
1 MiB-strided descriptors only reached ~52% of per-engine DMA bandwidth).

Chunks: head taper, 9 x 6 MiB body DMAs (48 KiB/partition descriptors),
tail taper, alternating between the two HWDGE queues (sync + scalar) so
both rings keep the 16 SDMA engines fed.

Compute per chunk: DVE segmented reduce over the innermost a=128 giving
acc[p, s_local] (bf16 in/out -> 2x perf mode); periodic x-multiply+reduce
partial stages overlap the stream; a tiny pairing matmul folds partition
pairs (2o, 2o+1) into ret[o].
"""

import sys

sys.path.insert(0, "/opt/trn_rl_repo")

import ml_dtypes
import numpy as np

import concourse.bass as bass
import concourse.mybir as mybir
import concourse.tile as tile
from concourse import bacc
from concourse.bass_utils import run_bass_kernel_spmd

BF16 = ml_dtypes.bfloat16

P = 128
O_PER_CORE = 64
N_CORES = 8
S = 4096
A = 128
COLS = O_PER_CORE * S * A // P  # 262144 per-partition row length (elements)
SLOC = 2048  # s-values covered by each partition

# Chunk schedule (cols each; 2048 cols = 0.5 MiB in bf16). Head/tail taper
# so the first reduce starts early and the last reduce is short.
CHUNKS = [8192, 16384] + [24576] * 9 + [8192, 4096, 2048, 2048]
assert sum(CHUNKS) == COLS
# After these chunk indices, run a partial x-multiply+reduce stage.
PARTIAL_AFTER = [2, 5, 8, 10, 12, 13, 14]
NPART = len(PARTIAL_AFTER)

# DRAM tensor per chunk-size class: name -> (n_chunks, cols)
SIZE_CLASSES = {
    "w2k": (2, 2048),
    "w4k": (1, 4096),
    "w8k": (2, 8192),
    "w16k": (1, 16384),
    "w24k": (9, 24576),
}
# chunk index -> (tensor name, slot within tensor)
CHUNK_SRC = (
    [("w8k", 0), ("w16k", 0)]
    + [("w24k", k) for k in range(9)]
    + [("w8k", 1), ("w4k", 0), ("w2k", 0), ("w2k", 1)]
)

_CACHED_NC = None


def _build_nc():
    nc = bacc.Bacc(
        "TRN2",
        target_bir_lowering=False,
        debug=False,
        num_devices=N_CORES,
    )
    wts = {
        name: nc.dram_tensor(
            name, [n * P, cols], mybir.dt.bfloat16, kind="ExternalInput"
        ).ap()
        for name, (n, cols) in SIZE_CLASSES.items()
    }
    xt = nc.dram_tensor("xt", [P, SLOC], mybir.dt.bfloat16, kind="ExternalInput").ap()
    pmat = nc.dram_tensor(
        "pmat", [P, O_PER_CORE], mybir.dt.float32, kind="ExternalInput"
    ).ap()
    out = nc.dram_tensor(
        "out", [O_PER_CORE, 1], mybir.dt.float32, kind="ExternalOutput"
    ).ap()

    with tile.TileContext(nc) as tc:
        with (
            tc.tile_pool(name="wp", bufs=3) as wp,
            tc.tile_pool(name="const", bufs=1) as constp,
            tc.tile_pool(name="accp", bufs=1) as accp,
            tc.tile_pool(name="psum", bufs=1, space="PSUM") as psp,
        ):
            acc = accp.tile([P, SLOC], mybir.dt.bfloat16)
            accx = accp.tile([P, SLOC], mybir.dt.bfloat16)
            vparts = accp.tile([P, NPART], mybir.dt.float32)
            xt_t = constp.tile([P, SLOC], mybir.dt.bfloat16)
            pm_t = constp.tile([P, O_PER_CORE], mybir.dt.float32)

            # Constants go via SWDGE so the HWDGE queues carry only the
            # weight stream.
            nc.gpsimd.dma_start(xt_t[:], xt[:])
            nc.gpsimd.dma_start(pm_t[:], pmat[:])

            coff = 0  # acc column offset (completed s-values)
            pstart = 0
            pi = 0
            for k, cols in enumerate(CHUNKS):
                wt = wp.tile([P, max(CHUNKS)], mybir.dt.bfloat16, tag="wt")
                nseg = cols // A
                name, slot = CHUNK_SRC[k]
                src = wts[name][slot * P : (slot + 1) * P, :]
                eng = nc.sync if k % 2 == 0 else nc.scalar
                eng.dma_start(wt[:, :cols], src)
                seg = wt[:, :cols].rearrange("p (n a) -> p n a", a=A)
                with nc.allow_low_precision(
                    reason="bf16 segment sums; fp32 partials keep output err ~1e-3"
                ):
                    nc.vector.tensor_reduce(
                        acc[:, coff : coff + nseg],
                        seg,
                        axis=mybir.AxisListType.X,
                        op=mybir.AluOpType.add,
                    )
                coff += nseg
                if k == PARTIAL_AFTER[pi]:
                    with nc.allow_low_precision(
                        reason="bf16 elementwise mul; fp32 partials downstream"
                    ):
                        nc.vector.tensor_mul(
                            accx[:, pstart:coff],
                            acc[:, pstart:coff],
                            xt_t[:, pstart:coff],
                        )
                    nc.vector.tensor_reduce(
                        vparts[:, pi : pi + 1],
                        accx[:, pstart:coff],
                        axis=mybir.AxisListType.X,
                        op=mybir.AluOpType.add,
                    )
                    pstart = coff
                    pi += 1
            assert coff == SLOC and pi == NPART

            v = accp.tile([P, 1], mybir.dt.float32)
            nc.vector.tensor_reduce(
                v[:], vparts[:], axis=mybir.AxisListType.X, op=mybir.AluOpType.add
            )
            ps = psp.tile([O_PER_CORE, 1], mybir.dt.float32)
            nc.tensor.matmul(ps[:], pm_t[:], v[:], start=True, stop=True)
            res = accp.tile([O_PER_CORE, 1], mybir.dt.float32)
            nc.scalar.copy(res[:], ps[:])
            nc.sync.dma_start(out[:], res[:])

    nc.compile()
    return nc


def _get_nc():
    global _CACHED_NC
    if _CACHED_NC is None:
        _CACHED_NC = _build_nc()
    return _CACHED_NC


def _in_maps(x, weights):
    x = np.ascontiguousarray(np.asarray(x, dtype=np.float32))
    weights = np.asarray(weights, dtype=np.float32)
    xt = np.tile(x.reshape(2, SLOC), (P // 2, 1)).astype(BF16)
    pmat = np.zeros((P, O_PER_CORE), dtype=np.float32)
    pmat[np.arange(P), np.arange(P) // 2] = 1.0

    # Column ranges per chunk in the logical [P, COLS] view.
    offs = np.cumsum([0] + CHUNKS)
    maps = []
    for c in range(N_CORES):
        wc = (
            weights[c * O_PER_CORE : (c + 1) * O_PER_CORE]
            .reshape(P, COLS)
            .astype(BF16)
        )
        m = {"xt": xt, "pmat": pmat}
        arrs = {
            name: np.empty((n * P, cols), dtype=BF16)
            for name, (n, cols) in SIZE_CLASSES.items()
        }
        for k, cols in enumerate(CHUNKS):
            name, slot = CHUNK_SRC[k]
            arrs[name][slot * P : (slot + 1) * P, :] = wc[:, offs[k] : offs[k + 1]]
        m.update(arrs)
        maps.append(m)
    return maps


def run(x, weights, trace=False):
    """Run on hardware; returns (ret[512], BassKernelResults)."""
    nc = _get_nc()
    res = run_bass_kernel_spmd(
        nc, _in_maps(x, weights), list(range(N_CORES)), trace=trace
    )
    ret = np.concatenate(
        [res.results[c]["out"].reshape(O_PER_CORE) for c in range(N_CORES)]
    ).astype(np.float32)
    return ret, res


def kernel(x, weights):
    ret, _ = run(x, weights)
    return ret


# revision 7
# speedup vs baseline: 2.7757x; 1.4235x over previous
"""Trainium2 Bass kernel for nn_LSH: ret[o] = sum_{s,a} x[s] * w[o,s,a].

x: [1, 4096] f32, weights: [512, 4096, 128] f32 -> ret: [512] f32.

PE-reduction variant: out_dim 512 is split 64-per-core across 8 cores and
weights are cast to bf16 on the host (2e-2 tolerance; halves HBM traffic).

Per core, weights are repacked so the TensorEngine does the whole
contraction: partitions = s within a 128-wide s-tile t (32 tiles), moving
columns = (o_local, a) for one 32-o half h.  A [128,1] stationary x-tile
makes matmul compute out[0, j] = sum_p x[t*128+p] * w[p, j]; accumulating
over the 32 s-tiles in 8 PSUM banks ([1,512] fp32 each) yields
v[o_l*128+a] = sum_s x[s] w[o,s,a] for the half.  DVE then segment-reduces
each bank over a (PSUM->SBUF) into ret[o].  PE consumes moving data at
~1 col/cycle (614 GB/s bf16), so the 64 MiB HBM stream is the only
bottleneck; DVE/PE tail work is ~10 us.

Stream layout: blocks of [128 s, 4096 (o_l,a)] bf16 (1 MiB), order
h-major then t; chunk DMAs are fully contiguous DRAM blocks (up to 6 MiB,
48 KiB/partition descriptors) alternating the two HWDGE queues.
"""

import sys

sys.path.insert(0, "/opt/trn_rl_repo")

import ml_dtypes
import numpy as np

import concourse.bass as bass
import concourse.mybir as mybir
import concourse.tile as tile
from concourse import bacc
from concourse.bass_utils import run_bass_kernel_spmd

BF16 = ml_dtypes.bfloat16

P = 128
O_PER_CORE = 64
N_CORES = 8
S = 4096
A = 128
NT = S // P  # 32 s-tiles
NH = 2  # column halves (32 o_locals each)
HCOLS = 32 * A  # 4096 moving cols per block
NBANK = 8
BANK = HCOLS // NBANK  # 512 cols per PSUM bank

# Chunk schedule in blocks (1 block = one s-tile's [128, 4096] = 1 MiB bf16),
# per half; head/tail taper. Chunks never span halves.
CHUNKS_H0 = [2, 4, 6, 6, 6, 6, 2]
CHUNKS_H1 = [6, 6, 6, 6, 4, 2, 1, 1]
assert sum(CHUNKS_H0) == NT and sum(CHUNKS_H1) == NT

# DRAM tensor per chunk-size class (in blocks): name -> (n_chunks, blocks)
SIZE_CLASSES = {
    "w1": (2, 1),
    "w2": (3, 2),
    "w4": (2, 4),
    "w6": (8, 6),
}
CHUNK_SRC = (
    [("w2", 0), ("w4", 0), ("w6", 0), ("w6", 1), ("w6", 2), ("w6", 3), ("w2", 1)]
    + [("w6", 4), ("w6", 5), ("w6", 6), ("w6", 7), ("w4", 1), ("w2", 2), ("w1", 0),
       ("w1", 1)]
)
CHUNKS = CHUNKS_H0 + CHUNKS_H1

_CACHED_NC = None


def _build_nc():
    nc = bacc.Bacc(
        "TRN2",
        target_bir_lowering=False,
        debug=False,
        num_devices=N_CORES,
    )
    wts = {
        name: nc.dram_tensor(
            name, [n * P, blocks * HCOLS], mybir.dt.bfloat16, kind="ExternalInput"
        ).ap()
        for name, (n, blocks) in SIZE_CLASSES.items()
    }
    xs = nc.dram_tensor("xs", [P, NT], mybir.dt.bfloat16, kind="ExternalInput").ap()
    out = nc.dram_tensor("out", [1, O_PER_CORE], mybir.dt.float32,
                         kind="ExternalOutput").ap()

    with tile.TileContext(nc) as tc:
        with (
            tc.tile_pool(name="wp", bufs=3) as wp,
            tc.tile_pool(name="const", bufs=1) as constp,
            tc.tile_pool(name="psum", bufs=1, space="PSUM") as psp,
        ):
            xs_t = constp.tile([P, NT], mybir.dt.bfloat16)
            rh = constp.tile([1, O_PER_CORE], mybir.dt.float32)
            ps = [
                psp.tile([1, BANK], mybir.dt.float32, name=f"ps{b}")
                for b in range(NBANK)
            ]

            # Constants go via SWDGE so the HWDGE queues carry only the
            # weight stream.
            nc.gpsimd.dma_start(xs_t[:], xs[:])

            ci = 0
            for h in range(NH):
                t = 0
                for blocks in (CHUNKS_H0 if h == 0 else CHUNKS_H1):
                    wt = wp.tile([P, 6 * HCOLS], mybir.dt.bfloat16, tag="wt")
                    cols = blocks * HCOLS
                    name, slot = CHUNK_SRC[ci]
                    src = wts[name][slot * P : (slot + 1) * P, :]
                    eng = nc.sync if ci % 2 == 0 else nc.scalar
                    eng.dma_start(wt[:, :cols], src)
                    for b_local in range(blocks):
                        for b in range(NBANK):
                            nc.tensor.matmul(
                                ps[b][:],
                                xs_t[:, t : t + 1],
                                wt[:, b_local * HCOLS + b * BANK :
                                   b_local * HCOLS + (b + 1) * BANK],
                                start=(t == 0),
                                stop=(t == NT - 1),
                                skip_group_check=True,
                            )
                        t += 1
                    ci += 1
                assert t == NT
                # Fold each bank over a: ps[b] [1, 4 o_l, 128 a] -> rh [1, 4]
                for b in range(NBANK):
                    seg = ps[b][:].rearrange("p (o a) -> p o a", a=A)
                    nc.vector.tensor_reduce(
                        rh[:, h * 32 + b * 4 : h * 32 + (b + 1) * 4],
                        seg,
                        axis=mybir.AxisListType.X,
                        op=mybir.AluOpType.add,
                    )
            nc.sync.dma_start(out[:], rh[:])

    nc.compile()
    return nc


def _get_nc():
    global _CACHED_NC
    if _CACHED_NC is None:
        _CACHED_NC = _build_nc()
    return _CACHED_NC


def _in_maps(x, weights):
    x = np.ascontiguousarray(np.asarray(x, dtype=np.float32))
    weights = np.asarray(weights, dtype=np.float32)
    # xs[p, t] = x[t*128 + p]
    xs = np.ascontiguousarray(x.reshape(NT, P).T).astype(BF16)

    maps = []
    for c in range(N_CORES):
        wc = weights[c * O_PER_CORE : (c + 1) * O_PER_CORE]  # [64, 4096, 128]
        # [h, o_l, t, p, a] -> [h, t, p, o_l, a] -> flat [64 blocks, 128, 4096]
        flat = (
            wc.reshape(NH, 32, NT, P, A)
            .transpose(0, 2, 3, 1, 4)
            .reshape(NH * NT, P, HCOLS)
            .astype(BF16)
        )
        m = {"xs": xs}
        arrs = {
            name: np.empty((n * P, blocks * HCOLS), dtype=BF16)
            for name, (n, blocks) in SIZE_CLASSES.items()
        }
        j = 0
        for ci, blocks in enumerate(CHUNKS):
            name, slot = CHUNK_SRC[ci]
            blk = flat[j : j + blocks]  # [blocks, 128, 4096]
            arrs[name][slot * P : (slot + 1) * P, :] = (
                blk.transpose(1, 0, 2).reshape(P, blocks * HCOLS)
            )
            j += blocks
        assert j == NH * NT
        m.update(arrs)
        maps.append(m)
    return maps


def run(x, weights, trace=False):
    """Run on hardware; returns (ret[512], BassKernelResults)."""
    nc = _get_nc()
    res = run_bass_kernel_spmd(
        nc, _in_maps(x, weights), list(range(N_CORES)), trace=trace
    )
    ret = np.concatenate(
        [res.results[c]["out"].reshape(O_PER_CORE) for c in range(N_CORES)]
    ).astype(np.float32)
    return ret, res


def kernel(x, weights):
    ret, _ = run(x, weights)
    return ret


# revision 8
# speedup vs baseline: 3.0320x; 1.0923x over previous
"""Trainium2 Bass kernel for nn_LSH: ret[o] = sum_{s,a} x[s] * w[o,s,a].

x: [1, 4096] f32, weights: [512, 4096, 128] f32 -> ret: [512] f32.

PE-reduction variant: out_dim 512 is split 64-per-core across 8 cores and
weights are cast to bf16 on the host (2e-2 tolerance; halves HBM traffic).

Per core, weights are repacked so the TensorEngine does the whole
contraction: partitions = s within a 128-wide s-tile t (32 tiles), moving
columns = (o_local, a) for one 16-o quarter q.  A [128,1] stationary
x-tile makes matmul compute out[0, j] = sum_p x[t*128+p] * w[p, j];
accumulating over the 32 s-tiles in 4 PSUM banks ([1,512] fp32 each)
yields v[o_l*128+a] = sum_s x[s] w[o,s,a] for the quarter.  Quarters
alternate between two 4-bank sets so the DVE drain (segment-reduce over a,
PSUM->SBUF) of quarter q overlaps accumulation of q+1.  PE consumes
moving data at ~1 col/cycle, so the 64 MiB HBM stream is the only
bottleneck.

Stream layout: blocks of [128 s, 2048 (o_l,a)] bf16 (0.5 MiB), order
q-major then t; chunk DMAs are contiguous DRAM blocks (up to 6 MiB).
Each chunk is fetched as TWO column-half DMAs issued on the two HWDGE
queues (sync + scalar) concurrently, halving per-chunk latency and
keeping both rings on the same chunk.
"""

import sys

sys.path.insert(0, "/opt/trn_rl_repo")

import ml_dtypes
import numpy as np

import concourse.bass as bass
import concourse.mybir as mybir
import concourse.tile as tile
from concourse import bacc
from concourse.bass_utils import run_bass_kernel_spmd

BF16 = ml_dtypes.bfloat16

P = 128
O_PER_CORE = 64
N_CORES = 8
S = 4096
A = 128
NT = S // P  # 32 s-tiles
NQ = 4  # o quarters (16 o_locals each)
QCOLS = 16 * A  # 2048 moving cols per block
NBANK = 4  # PSUM banks per quarter (two alternating sets)
BANK = QCOLS // NBANK  # 512 cols per PSUM bank
NBLK = NQ * NT  # 128 stream blocks

# Chunk schedule in blocks (1 block = 0.5 MiB bf16); head/tail taper.
CHUNKS = [2, 4, 8] + [12] * 9 + [4, 2]
assert sum(CHUNKS) == NBLK

# DRAM tensor per chunk-size class (in blocks): name -> (n_chunks, blocks)
SIZE_CLASSES = {
    "wa": (2, 2),
    "wb": (2, 4),
    "wc": (1, 8),
    "wd": (9, 12),
}
CHUNK_SRC = (
    [("wa", 0), ("wb", 0), ("wc", 0)]
    + [("wd", k) for k in range(9)]
    + [("wb", 1), ("wa", 1)]
)

_CACHED_NC = None


def _build_nc():
    nc = bacc.Bacc(
        "TRN2",
        target_bir_lowering=False,
        debug=False,
        num_devices=N_CORES,
    )
    wts = {
        name: nc.dram_tensor(
            name, [n * P, blocks * QCOLS], mybir.dt.bfloat16, kind="ExternalInput"
        ).ap()
        for name, (n, blocks) in SIZE_CLASSES.items()
    }
    xs = nc.dram_tensor("xs", [P, NT], mybir.dt.bfloat16, kind="ExternalInput").ap()
    out = nc.dram_tensor("out", [1, O_PER_CORE], mybir.dt.float32,
                         kind="ExternalOutput").ap()

    with tile.TileContext(nc) as tc:
        with (
            tc.tile_pool(name="wp", bufs=3) as wp,
            tc.tile_pool(name="const", bufs=1) as constp,
            tc.tile_pool(name="psum", bufs=1, space="PSUM") as psp,
        ):
            xs_t = constp.tile([P, NT], mybir.dt.bfloat16)
            rh = constp.tile([1, O_PER_CORE], mybir.dt.float32)
            ps = [
                psp.tile([1, BANK], mybir.dt.float32, name=f"ps{b}")
                for b in range(2 * NBANK)
            ]

            # Constants go via SWDGE so the HWDGE queues carry only the
            # weight stream.
            nc.gpsimd.dma_start(xs_t[:], xs[:])

            blk = 0  # global block index (q = blk // NT, t = blk % NT)
            for ci, blocks in enumerate(CHUNKS):
                wt = wp.tile([P, max(CHUNKS) * QCOLS], mybir.dt.bfloat16, tag="wt")
                cols = blocks * QCOLS
                name, slot = CHUNK_SRC[ci]
                src = wts[name][slot * P : (slot + 1) * P, :]
                half = cols // 2
                nc.sync.dma_start(wt[:, :half], src[:, :half])
                nc.scalar.dma_start(wt[:, half:cols], src[:, half:])
                for b_local in range(blocks):
                    q, t = blk // NT, blk % NT
                    bankset = (q % 2) * NBANK
                    for j in range(NBANK):
                        nc.tensor.matmul(
                            ps[bankset + j][:],
                            xs_t[:, t : t + 1],
                            wt[:, b_local * QCOLS + j * BANK :
                               b_local * QCOLS + (j + 1) * BANK],
                            start=(t == 0),
                            stop=(t == NT - 1),
                            skip_group_check=True,
                        )
                    if t == NT - 1:
                        # Quarter done: fold each bank over a into ret.
                        for j in range(NBANK):
                            seg = ps[bankset + j][:].rearrange(
                                "p (o a) -> p o a", a=A
                            )
                            nc.vector.tensor_reduce(
                                rh[:, q * 16 + j * 4 : q * 16 + (j + 1) * 4],
                                seg,
                                axis=mybir.AxisListType.X,
                                op=mybir.AluOpType.add,
                            )
                    blk += 1
            assert blk == NBLK
            nc.sync.dma_start(out[:], rh[:])

    nc.compile()
    return nc


def _get_nc():
    global _CACHED_NC
    if _CACHED_NC is None:
        _CACHED_NC = _build_nc()
    return _CACHED_NC


def _in_maps(x, weights):
    x = np.ascontiguousarray(np.asarray(x, dtype=np.float32))
    weights = np.asarray(weights, dtype=np.float32)
    # xs[p, t] = x[t*128 + p]
    xs = np.ascontiguousarray(x.reshape(NT, P).T).astype(BF16)

    maps = []
    for c in range(N_CORES):
        wc = weights[c * O_PER_CORE : (c + 1) * O_PER_CORE]  # [64, 4096, 128]
        # [q, o_l, t, p, a] -> [q, t, p, o_l, a] -> flat [128 blocks, 128, 2048]
        flat = (
            wc.reshape(NQ, 16, NT, P, A)
            .transpose(0, 2, 3, 1, 4)
            .reshape(NBLK, P, QCOLS)
            .astype(BF16)
        )
        m = {"xs": xs}
        arrs = {
            name: np.empty((n * P, blocks * QCOLS), dtype=BF16)
            for name, (n, blocks) in SIZE_CLASSES.items()
        }
        j = 0
        for ci, blocks in enumerate(CHUNKS):
            name, slot = CHUNK_SRC[ci]
            arrs[name][slot * P : (slot + 1) * P, :] = (
                flat[j : j + blocks].transpose(1, 0, 2).reshape(P, blocks * QCOLS)
            )
            j += blocks
        assert j == NBLK
        m.update(arrs)
        maps.append(m)
    return maps


def run(x, weights, trace=False):
    """Run on hardware; returns (ret[512], BassKernelResults)."""
    nc = _get_nc()
    res = run_bass_kernel_spmd(
        nc, _in_maps(x, weights), list(range(N_CORES)), trace=trace
    )
    ret = np.concatenate(
        [res.results[c]["out"].reshape(O_PER_CORE) for c in range(N_CORES)]
    ).astype(np.float32)
    return ret, res


def kernel(x, weights):
    ret, _ = run(x, weights)
    return ret
